# revision 26
# baseline (speedup 1.0000x reference)
"""Trainium2 Bass kernel for CrossAttentionComplexQ.

Shapes: q_real (64, 8, 256), kv (64, 4096, 512) -> out (64, 8, 256).

Math (per batch b):
    q  = complex-rotate(q_real, freq)           # rope-style pairwise rotation
    qn = LayerNorm(q) * ln_w + ln_b
    k  = kv @ Wk.T ; v = kv @ Wv.T
    out = ((qn @ k.T) / 16) @ v

Since there is no softmax the whole chain is linear in kv, so per batch:
    out = qk' @ G @ Wv.T    with qk' = qn @ Wk / 16 (8, 512)
                            and  G = kv.T @ kv      (512, 512 Gram matrix)
G only ever contracts over the sequence dim s, so kv is consumed in its
natural [s-partition, d-free] SBUF layout -- no transposes of the big
tensor. G is symmetric: only the 10 upper-triangle 128x128 blocks are
computed; mirrors come from cheap 128x128 PE transposes.

kv is cast to fp8e4m3 on host and the Gram runs in DoubleRow perf mode
(256-deep contraction per instruction, 0.5 PE cycles per output column
-- 4x the bf16 column rate per MAC). That moves the kernel from
PE-bound (~153us busy) to DMA-bound (~47us for 16.8MB/core at
~360GB/s). The Gram is permutation-invariant over s, so the host
pre-permutes kv into a layout where every DMA line is 4KB contiguous.
kv is fully SBUF-resident (128KB/partition) and all DMA is issued up
front so the 16 rings stay saturated. Gram for batch b+1 is emitted
before batch b's mirror/FT tail so the PE never idles on vector
copies; the final FT @ Wv.T is batched across all 8 local batches.

Sharding: pure data parallel, batch dim 64 -> 8 batches per NeuronCore.
rope/LN/q-projection run on device in fp32/bf16.
"""

import sys
import numpy as np
import ml_dtypes

for _p in ("/opt/trn_rl_repo",):
    if _p not in sys.path:
        sys.path.append(_p)

B, SQ, SKV, DQ, DKV = 64, 8, 4096, 256, 512
NCORES = 8
BL = B // NCORES          # local batches per core
R = BL * SQ               # query rows per core
LN_EPS = 1e-5
SCALE = 1.0 / (DQ ** 0.5)
NCH = 16                  # 256-row s-chunks per batch (DoubleRow pairs)
NG = 4                    # kv DMA groups per batch (4KB partition lines)
GCH = NCH // NG           # chunks per DMA group

_CACHE = {}


def _build():
    import concourse.mybir as mybir
    import concourse.tile as tile
    from concourse import bacc
    from concourse.masks import make_identity

    dt = mybir.dt
    f32, bf16, f8 = dt.float32, dt.bfloat16, dt.float8e4
    DR = mybir.MatmulPerfMode.DoubleRow

    nc = bacc.Bacc("TRN2", target_bir_lowering=False, debug=False,
                   num_devices=NCORES)
    q_d = nc.dram_tensor("q", (R, DQ), f32, kind="ExternalInput")
    qs_d = nc.dram_tensor("qsw", (R, DQ), f32, kind="ExternalInput")
    kv_d = nc.dram_tensor("kv", (BL, NG, 128, GCH, 2, DKV), f8,
                          kind="ExternalInput")
    c_d = nc.dram_tensor("cosf", (R, DQ), f32, kind="ExternalInput")
    s_d = nc.dram_tensor("sinf", (R, DQ), f32, kind="ExternalInput")
    wk_d = nc.dram_tensor("wk", (128, 2, DKV), bf16, kind="ExternalInput")
    bk_d = nc.dram_tensor("bk", (128, 4), f32, kind="ExternalInput")
    wv_d = nc.dram_tensor("wv", (128, 4, DQ), bf16, kind="ExternalInput")
    out_d = nc.dram_tensor("out", (R, DQ), f32, kind="ExternalOutput")

    with tile.TileContext(nc) as tc:
        with (
            tc.tile_pool(name="const", bufs=1) as const,
            tc.tile_pool(name="qstage", bufs=1) as qpool,
            tc.tile_pool(name="gsb", bufs=2) as gpool,
            tc.tile_pool(name="small", bufs=2) as spool,
            tc.tile_pool(name="psg", bufs=2, space="PSUM") as psg,
            tc.tile_pool(name="pss", bufs=2, space="PSUM") as pss,
        ):
            # kv fully resident: [part, b, chunk, pair, d], 128KB/partition.
            # batch-0 groups go on the queue first so Gram 0 starts early;
            # the small q/weight DMAs are next so the LN chain and qkT are
            # ready long before batch 0's FT; the rest of kv follows.
            kv_sb = const.tile([128, BL, NCH, 2, DKV], f8)

            def dma_group(b, g):
                nc.sync.dma_start(kv_sb[:, b, GCH * g:GCH * (g + 1), :, :],
                                  kv_d[b, g])

            # batch 0 in graded pieces so the first Gram matmuls start as
            # early as possible
            nc.sync.dma_start(kv_sb[:, 0, 0:1, :, :], kv_d[0, 0, :, 0:1])
            nc.sync.dma_start(kv_sb[:, 0, 1:2, :, :], kv_d[0, 0, :, 1:2])
            nc.sync.dma_start(kv_sb[:, 0, 2:4, :, :], kv_d[0, 0, :, 2:4])
            for g in range(1, NG):
                dma_group(0, g)

            ident = const.tile([128, 128], bf16)
            make_identity(nc, ident[:])

            wk_sb = const.tile([128, 2, DKV], bf16)
            nc.sync.dma_start(wk_sb[:], wk_d[:])
            bk_sb = const.tile([128, 4], f32)
            nc.sync.dma_start(bk_sb[:], bk_d[:])
            wv_sb = const.tile([128, 4, DQ], bf16)
            nc.sync.dma_start(wv_sb[:], wv_d[:])

            q_sb = qpool.tile([R, DQ], f32)
            nc.sync.dma_start(q_sb[:], q_d[:])
            qs_sb = qpool.tile([R, DQ], f32)
            nc.sync.dma_start(qs_sb[:], qs_d[:])
            c_sb = qpool.tile([R, DQ], f32)
            nc.sync.dma_start(c_sb[:], c_d[:])
            s_sb = qpool.tile([R, DQ], f32)
            nc.sync.dma_start(s_sb[:], s_d[:])

            # per-group dma_starts: ring assignment is round-robin per
            # dma_start, so ~32 starts keep all 16 DMA rings busy.
            for b in range(1, BL):
                for g in range(NG):
                    dma_group(b, g)

            # ---- q stage: rope + LayerNorm, all 64 rows (vector/scalar) ----
            qrot = qpool.tile([R, DQ], f32)
            m2 = qpool.tile([R, DQ], f32)
            nc.vector.tensor_mul(qrot[:], q_sb[:], c_sb[:])
            nc.vector.tensor_mul(m2[:], qs_sb[:], s_sb[:])
            nc.vector.tensor_add(qrot[:], qrot[:], m2[:])

            rsum = qpool.tile([R, 1], f32)
            nc.vector.tensor_reduce(rsum[:], qrot[:], mybir.AxisListType.X,
                                    mybir.AluOpType.add)
            mu = qpool.tile([R, 1], f32)
            nc.vector.tensor_scalar_mul(mu[:], rsum[:], 1.0 / DQ)
            xm = qpool.tile([R, DQ], f32)
            nc.vector.tensor_scalar_sub(xm[:], qrot[:], mu[:])
            sq = qpool.tile([R, DQ], f32)
            nc.vector.tensor_mul(sq[:], xm[:], xm[:])
            vsum = qpool.tile([R, 1], f32)
            nc.vector.tensor_reduce(vsum[:], sq[:], mybir.AxisListType.X,
                                    mybir.AluOpType.add)
            eps = qpool.tile([R, 1], f32)
            nc.gpsimd.memset(eps[:], LN_EPS)
            std = qpool.tile([R, 1], f32)
            nc.scalar.activation(std[:], vsum[:],
                                 mybir.ActivationFunctionType.Sqrt,
                                 bias=eps[:], scale=1.0 / DQ)
            rstd = qpool.tile([R, 1], f32)
            nc.vector.reciprocal(rstd[:], std[:])
            qhat = qpool.tile([R, DQ], bf16)
            nc.vector.tensor_scalar_mul(qhat[:], xm[:], rstd[:])

            # q-projection PE work is emitted after batch 1's Gram: the PE
            # starts on Gram as soon as kv lands instead of stalling on the
            # LN chain. qkT is ready well before batch 0's FT needs it.
            qhatT = const.tile([128, 2, R], bf16)
            qkT = const.tile([128, 4, R], bf16)

            def emit_qproj():
                for ch in range(2):
                    tps = pss.tile([128, R], bf16, tag="scratch")
                    nc.tensor.transpose(tps[:], qhat[:, 128 * ch:128 * (ch + 1)],
                                        ident[:R, :R])
                    nc.vector.tensor_copy(qhatT[:, ch, :], tps[:])
                for j in range(4):
                    ps = pss.tile([128, R], f32, tag="scratch")
                    for ch in range(2):
                        nc.tensor.matmul(ps[:], wk_sb[:, ch, 128 * j:128 * (j + 1)],
                                         qhatT[:, ch, :],
                                         start=(ch == 0), stop=(ch == 1))
                    nc.vector.tensor_scalar_add(qkT[:, j, :], ps[:],
                                                bk_sb[:, j:j + 1])

            # ---- per-batch: Gram triangle (fp8 DoubleRow) + FT tail ----
            # ga holds G row-block 0 (512 cols); gb packs row-block 1
            # (cols 128:512 -> 0:384) and row-block 3 (384:512) in one
            # PSUM bank; gc holds row-block 2 (256:512 -> 0:256).
            # start marks the whole bank pending-zero, so only the FIRST
            # mm into a bank carries start and only the LAST one stop.
            gtiles = {}

            def emit_gram(b):
                ga = psg.tile([128, 512], f32, tag="GA")
                gb = psg.tile([128, 512], f32, tag="GB")
                gc = psg.tile([128, 256], f32, tag="GC")
                gtiles[b] = (ga, gb, gc)
                for ch in range(NCH):
                    st, sp = (ch == 0), (ch == NCH - 1)
                    kt = kv_sb[:, b, ch]          # [128, 2, 512] fp8
                    nc.tensor.matmul(ga[:, 0:256], kt[:, :, 0:128],
                                     kt[:, :, 0:256], perf_mode=DR,
                                     start=st, stop=False)
                    nc.tensor.matmul(ga[:, 256:512], kt[:, :, 0:128],
                                     kt[:, :, 256:512], perf_mode=DR,
                                     start=False, stop=sp)
                    nc.tensor.matmul(gb[:, 0:256], kt[:, :, 128:256],
                                     kt[:, :, 128:384], perf_mode=DR,
                                     start=st, stop=False)
                    nc.tensor.matmul(gb[:, 256:384], kt[:, :, 128:256],
                                     kt[:, :, 384:512], perf_mode=DR,
                                     start=False, stop=False)
                    nc.tensor.matmul(gc[:, 0:256], kt[:, :, 256:384],
                                     kt[:, :, 256:512], perf_mode=DR,
                                     start=st, stop=sp)
                    nc.tensor.matmul(gb[:, 384:512], kt[:, :, 384:512],
                                     kt[:, :, 384:512], perf_mode=DR,
                                     start=False, stop=sp)

            ft_all = const.tile([128, 4, R], bf16)

            def emit_tail(b):
                ga, gb, gc = gtiles.pop(b)
                # Gsb full rows [128, row k, d2] from triangle + transposes;
                # copies split across scalar and vector so they drain in
                # parallel under batch b+1's Gram.
                gsb = gpool.tile([128, 4, DKV], bf16, tag="gsb")
                nc.scalar.copy(gsb[:, 0, :], ga[:, :])
                nc.vector.tensor_copy(gsb[:, 1, 128:512], gb[:, 0:384])
                nc.scalar.copy(gsb[:, 2, 256:512], gc[:, 0:256])
                nc.vector.tensor_copy(gsb[:, 3, 384:512], gb[:, 384:512])
                # FT[d2, i] = sum_d1 G[d1, d2] qkT[d1, i]   (G symmetric)
                # j=3 reads only upper blocks, so it runs ahead of the
                # mirror transposes and copies.
                ftp = pss.tile([128, 4, SQ], f32, tag="scratch")

                def ft(j):
                    for k in range(4):
                        nc.tensor.matmul(ftp[:, j, :],
                                         gsb[:, k, 128 * j:128 * (j + 1)],
                                         qkT[:, k, SQ * b:SQ * (b + 1)],
                                         start=(k == 0), stop=(k == 3))

                ft(3)

                # all 6 mirror transposes packed into one PSUM tile so they
                # run back-to-back on the PE (pending-zero start/stop trick),
                # grouped by destination row so they copy back in 3 strided
                # vector copies. Row-3 mirrors first: FT j=2,1,0 need them.
                tps = pss.tile([128, 6, 128], bf16, tag="scratch")
                for i, (src_row, src_col) in enumerate(
                        ((0, 3), (1, 3), (2, 3), (0, 2), (1, 2), (0, 1))):
                    nc.tensor.matmul(
                        tps[:, i, :],
                        gsb[:, src_row, 128 * src_col:128 * (src_col + 1)],
                        ident[:], is_transpose=True,
                        start=(i == 0), stop=(i == 5))
                nc.vector.tensor_copy(gsb[:, 3, 0:384], tps[:, 0:3, :])
                nc.vector.tensor_copy(gsb[:, 2, 0:256], tps[:, 3:5, :])
                nc.vector.tensor_copy(gsb[:, 1, 0:128], tps[:, 5, :])

                for j in (2, 1, 0):
                    ft(j)
                nc.vector.tensor_copy(ft_all[:, :, SQ * b:SQ * (b + 1)], ftp[:])

            # out[(b,i), q] = sum_dk FT[dk, (b,i)] WvT[dk, q], in two halves
            # so the first half's matmul + DMA hide under later Gram work
            # and only the second half sits in the drain.
            out_sb = qpool.tile([R, DQ], f32)

            def emit_out(r0, r1):
                rows = slice(r0, r1)
                outp = pss.tile([r1 - r0, DQ], f32, tag="scratch")
                for j in range(4):
                    nc.tensor.matmul(outp[:], ft_all[:, j, rows],
                                     wv_sb[:, j, :],
                                     start=(j == 0), stop=(j == 3))
                nc.vector.tensor_copy(out_sb[rows, :], outp[:])
                nc.sync.dma_start(out_d[rows, :], out_sb[rows, :])

            # software pipeline: Gram b+1 runs on the PE while batch b's
            # G copies drain on the vector engine.
            emit_gram(0)
            for b in range(BL):
                if b + 1 < BL:
                    emit_gram(b + 1)
                if b == 0:
                    emit_qproj()
                emit_tail(b)
                if b == 4:
                    emit_out(0, 32)
            emit_out(32, 64)

    nc.compile()
    return nc


def _get_nc():
    if "nc" not in _CACHE:
        _CACHE["nc"] = _build()
    return _CACHE["nc"]


def _prep_inputs(q_real, kv, freq_cos, freq_sin, ln_w, ln_b, Wk, Wv):
    f32 = np.float32
    bf16 = ml_dtypes.bfloat16
    f8 = ml_dtypes.float8_e4m3
    q_real = np.asarray(q_real, f32)
    kv = np.asarray(kv, f32)
    freq_cos = np.asarray(freq_cos, f32)
    freq_sin = np.asarray(freq_sin, f32)
    ln_w = np.asarray(ln_w, f32)
    ln_b = np.asarray(ln_b, f32)
    Wk = np.asarray(Wk, f32)
    Wv = np.asarray(Wv, f32)

    # interleaved cos/sin patterns with rotation signs folded in
    C = np.empty((SQ, DQ), f32)
    C[:, 0::2] = freq_cos
    C[:, 1::2] = freq_cos
    S = np.empty((SQ, DQ), f32)
    S[:, 0::2] = -freq_sin
    S[:, 1::2] = freq_sin
    C = np.tile(C, (BL, 1))
    S = np.tile(S, (BL, 1))

    # pair-swapped q (pure layout shuffle; rotation math runs on device)
    qsw = np.empty_like(q_real)
    qsw[..., 0::2] = q_real[..., 1::2]
    qsw[..., 1::2] = q_real[..., 0::2]

    # fold ln_w and the 1/sqrt(dq) score scale into Wk; ln_b becomes a bias
    wk_f = (ln_w[:, None] * Wk) * SCALE           # (256, 512)
    bk = (ln_b @ Wk) * SCALE                      # (512,)
    wk_arr = np.ascontiguousarray(
        wk_f.reshape(2, 128, DKV).transpose(1, 0, 2)).astype(bf16)
    bk_arr = np.ascontiguousarray(bk.reshape(4, 128).T).astype(f32)
    wv_arr = np.ascontiguousarray(
        Wv.T.reshape(4, 128, DQ).transpose(1, 0, 2)).astype(bf16)

    # fp8 kv, permuted so each DMA group is one contiguous block with 4KB
    # partition lines: s = ((g*GCH + gs)*2 + two)*128 + p. The Gram sums
    # over all s, so any fixed permutation of s is exact.
    kv8 = kv.astype(f8)
    kv8 = kv8.reshape(B, NG, GCH, 2, 128, DKV).transpose(0, 1, 4, 2, 3, 5)
    kv8 = np.ascontiguousarray(kv8)               # (B, NG, 128, GCH, 2, DKV)

    in_maps = []
    for c in range(NCORES):
        sl = slice(BL * c, BL * (c + 1))
        in_maps.append({
            "q": np.ascontiguousarray(q_real[sl].reshape(R, DQ)),
            "qsw": np.ascontiguousarray(qsw[sl].reshape(R, DQ)),
            "kv": kv8[sl],
            "cosf": C,
            "sinf": S,
            "wk": wk_arr,
            "bk": bk_arr,
            "wv": wv_arr,
        })
    return in_maps


def kernel(**inputs):
    from concourse.bass_utils import run_bass_kernel_spmd

    nc = _get_nc()
    in_maps = _prep_inputs(**inputs)
    res = run_bass_kernel_spmd(nc, in_maps, list(range(NCORES)))
    out = np.concatenate(
        [res.results[c]["out"].reshape(BL, SQ, DQ) for c in range(NCORES)], axis=0)
    return np.ascontiguousarray(out.astype(np.float32))


# revision 29
# speedup vs baseline: 1.0098x; 1.0098x over previous
"""Trainium2 Bass kernel for CrossAttentionComplexQ.

Shapes: q_real (64, 8, 256), kv (64, 4096, 512) -> out (64, 8, 256).

Math (per batch b):
    q  = complex-rotate(q_real, freq)           # rope-style pairwise rotation
    qn = LayerNorm(q) * ln_w + ln_b
    k  = kv @ Wk.T ; v = kv @ Wv.T
    out = ((qn @ k.T) / 16) @ v

Since there is no softmax the whole chain is linear in kv, so per batch:
    out = qk' @ G @ Wv.T    with qk' = qn @ Wk / 16 (8, 512)
                            and  G = kv.T @ kv      (512, 512 Gram matrix)
G only ever contracts over the sequence dim s, so kv is consumed in its
natural [s-partition, d-free] SBUF layout -- no transposes of the big
tensor. G is symmetric: only the 10 upper-triangle 128x128 blocks are
computed; mirrors come from cheap 128x128 PE transposes.

kv is cast to fp8e4m3 on host and the Gram runs in DoubleRow perf mode
(256-deep contraction per instruction, 0.5 PE cycles per output column
-- 4x the bf16 column rate per MAC). That moves the kernel from
PE-bound (~153us busy) to DMA-bound (~47us for 16.8MB/core at
~360GB/s). The Gram is permutation-invariant over s, so the host
pre-permutes kv into a layout where every DMA line is 4KB contiguous.
kv is fully SBUF-resident (128KB/partition) and all DMA is issued up
front so the 16 rings stay saturated. Gram for batch b+1 is emitted
before batch b's mirror/FT tail so the PE never idles on vector
copies; the final FT @ Wv.T is batched across all 8 local batches.

Sharding: pure data parallel, batch dim 64 -> 8 batches per NeuronCore.
rope/LN/q-projection run on device in fp32/bf16.
"""

import sys
import numpy as np
import ml_dtypes

for _p in ("/opt/trn_rl_repo",):
    if _p not in sys.path:
        sys.path.append(_p)

B, SQ, SKV, DQ, DKV = 64, 8, 4096, 256, 512
NCORES = 8
BL = B // NCORES          # local batches per core
R = BL * SQ               # query rows per core
LN_EPS = 1e-5
SCALE = 1.0 / (DQ ** 0.5)
NCH = 16                  # 256-row s-chunks per batch (DoubleRow pairs)
NG = 4                    # kv DMA groups per batch (4KB partition lines)
GCH = NCH // NG           # chunks per DMA group

_CACHE = {}


def _build():
    import concourse.mybir as mybir
    import concourse.tile as tile
    from concourse import bacc
    from concourse.masks import make_identity

    dt = mybir.dt
    f32, bf16, f8 = dt.float32, dt.bfloat16, dt.float8e4
    DR = mybir.MatmulPerfMode.DoubleRow

    nc = bacc.Bacc("TRN2", target_bir_lowering=False, debug=False,
                   num_devices=NCORES)
    q_d = nc.dram_tensor("q", (R, DQ), f32, kind="ExternalInput")
    qs_d = nc.dram_tensor("qsw", (R, DQ), f32, kind="ExternalInput")
    kv_d = nc.dram_tensor("kv", (BL, NG, 128, GCH, 2, DKV), f8,
                          kind="ExternalInput")
    c_d = nc.dram_tensor("cosf", (R, DQ), f32, kind="ExternalInput")
    s_d = nc.dram_tensor("sinf", (R, DQ), f32, kind="ExternalInput")
    wk_d = nc.dram_tensor("wk", (128, 2, DKV), bf16, kind="ExternalInput")
    bk_d = nc.dram_tensor("bk", (128, 4), f32, kind="ExternalInput")
    wv_d = nc.dram_tensor("wv", (128, 4, DQ), bf16, kind="ExternalInput")
    out_d = nc.dram_tensor("out", (R, DQ), f32, kind="ExternalOutput")

    with tile.TileContext(nc) as tc:
        with (
            tc.tile_pool(name="const", bufs=1) as const,
            tc.tile_pool(name="qstage", bufs=1) as qpool,
            tc.tile_pool(name="gsb", bufs=2) as gpool,
            tc.tile_pool(name="small", bufs=2) as spool,
            tc.tile_pool(name="psg", bufs=2, space="PSUM") as psg,
            tc.tile_pool(name="pss", bufs=2, space="PSUM") as pss,
        ):
            # kv fully resident: [part, b, chunk, pair, d], 128KB/partition.
            # batch-0 groups go on the queue first so Gram 0 starts early;
            # the small q/weight DMAs are next so the LN chain and qkT are
            # ready long before batch 0's FT; the rest of kv follows.
            kv_sb = const.tile([128, BL, NCH, 2, DKV], f8)

            def dma_group(b, g):
                nc.sync.dma_start(kv_sb[:, b, GCH * g:GCH * (g + 1), :, :],
                                  kv_d[b, g])

            # batch 0 in graded pieces so the first Gram matmuls start as
            # early as possible
            nc.sync.dma_start(kv_sb[:, 0, 0:2, :, :], kv_d[0, 0, :, 0:2])
            nc.sync.dma_start(kv_sb[:, 0, 2:4, :, :], kv_d[0, 0, :, 2:4])
            for g in range(1, NG):
                dma_group(0, g)

            ident = const.tile([128, 128], bf16)
            make_identity(nc, ident[:])

            wk_sb = const.tile([128, 2, DKV], bf16)
            nc.sync.dma_start(wk_sb[:], wk_d[:])
            bk_sb = const.tile([128, 4], f32)
            nc.sync.dma_start(bk_sb[:], bk_d[:])
            wv_sb = const.tile([128, 4, DQ], bf16)
            nc.sync.dma_start(wv_sb[:], wv_d[:])

            q_sb = qpool.tile([R, DQ], f32)
            nc.sync.dma_start(q_sb[:], q_d[:])
            qs_sb = qpool.tile([R, DQ], f32)
            nc.sync.dma_start(qs_sb[:], qs_d[:])
            c_sb = qpool.tile([R, DQ], f32)
            nc.sync.dma_start(c_sb[:], c_d[:])
            s_sb = qpool.tile([R, DQ], f32)
            nc.sync.dma_start(s_sb[:], s_d[:])

            # per-group dma_starts: ring assignment is round-robin per
            # dma_start, so ~32 starts keep all 16 DMA rings busy.
            for b in range(1, BL):
                for g in range(NG):
                    dma_group(b, g)

            # ---- q stage: rope + LayerNorm, all 64 rows (vector/scalar) ----
            qrot = qpool.tile([R, DQ], f32)
            m2 = qpool.tile([R, DQ], f32)
            nc.vector.tensor_mul(qrot[:], q_sb[:], c_sb[:])
            nc.vector.tensor_mul(m2[:], qs_sb[:], s_sb[:])
            nc.vector.tensor_add(qrot[:], qrot[:], m2[:])

            rsum = qpool.tile([R, 1], f32)
            nc.vector.tensor_reduce(rsum[:], qrot[:], mybir.AxisListType.X,
                                    mybir.AluOpType.add)
            mu = qpool.tile([R, 1], f32)
            nc.vector.tensor_scalar_mul(mu[:], rsum[:], 1.0 / DQ)
            xm = qpool.tile([R, DQ], f32)
            nc.vector.tensor_scalar_sub(xm[:], qrot[:], mu[:])
            sq = qpool.tile([R, DQ], f32)
            nc.vector.tensor_mul(sq[:], xm[:], xm[:])
            vsum = qpool.tile([R, 1], f32)
            nc.vector.tensor_reduce(vsum[:], sq[:], mybir.AxisListType.X,
                                    mybir.AluOpType.add)
            eps = qpool.tile([R, 1], f32)
            nc.gpsimd.memset(eps[:], LN_EPS)
            std = qpool.tile([R, 1], f32)
            nc.scalar.activation(std[:], vsum[:],
                                 mybir.ActivationFunctionType.Sqrt,
                                 bias=eps[:], scale=1.0 / DQ)
            rstd = qpool.tile([R, 1], f32)
            nc.vector.reciprocal(rstd[:], std[:])
            qhat = qpool.tile([R, DQ], bf16)
            nc.vector.tensor_scalar_mul(qhat[:], xm[:], rstd[:])

            # q-projection PE work is emitted after batch 1's Gram: the PE
            # starts on Gram as soon as kv lands instead of stalling on the
            # LN chain. qkT is ready well before batch 0's FT needs it.
            qhatT = const.tile([128, 2, R], bf16)
            qkT = const.tile([128, 4, R], bf16)

            def emit_qproj():
                for ch in range(2):
                    tps = pss.tile([128, R], bf16, tag="scratch")
                    nc.tensor.transpose(tps[:], qhat[:, 128 * ch:128 * (ch + 1)],
                                        ident[:R, :R])
                    nc.vector.tensor_copy(qhatT[:, ch, :], tps[:])
                for j in range(4):
                    ps = pss.tile([128, R], f32, tag="scratch")
                    for ch in range(2):
                        nc.tensor.matmul(ps[:], wk_sb[:, ch, 128 * j:128 * (j + 1)],
                                         qhatT[:, ch, :],
                                         start=(ch == 0), stop=(ch == 1))
                    nc.vector.tensor_scalar_add(qkT[:, j, :], ps[:],
                                                bk_sb[:, j:j + 1])

            # ---- per-batch: Gram triangle (fp8 DoubleRow) + FT tail ----
            # ga holds G row-block 0 (512 cols); gb packs row-block 1
            # (cols 128:512 -> 0:384) and row-block 3 (384:512) in one
            # PSUM bank; gc holds row-block 2 (256:512 -> 0:256).
            # start marks the whole bank pending-zero, so only the FIRST
            # mm into a bank carries start and only the LAST one stop.
            gtiles = {}

            def emit_gram(b):
                ga = psg.tile([128, 512], f32, tag="GA")
                gb = psg.tile([128, 512], f32, tag="GB")
                gc = psg.tile([128, 256], f32, tag="GC")
                gtiles[b] = (ga, gb, gc)
                for ch in range(NCH):
                    st, sp = (ch == 0), (ch == NCH - 1)
                    kt = kv_sb[:, b, ch]          # [128, 2, 512] fp8
                    nc.tensor.matmul(ga[:, 0:256], kt[:, :, 0:128],
                                     kt[:, :, 0:256], perf_mode=DR,
                                     start=st, stop=False)
                    nc.tensor.matmul(ga[:, 256:512], kt[:, :, 0:128],
                                     kt[:, :, 256:512], perf_mode=DR,
                                     start=False, stop=sp)
                    nc.tensor.matmul(gb[:, 0:256], kt[:, :, 128:256],
                                     kt[:, :, 128:384], perf_mode=DR,
                                     start=st, stop=False)
                    nc.tensor.matmul(gb[:, 256:384], kt[:, :, 128:256],
                                     kt[:, :, 384:512], perf_mode=DR,
                                     start=False, stop=False)
                    nc.tensor.matmul(gc[:, 0:256], kt[:, :, 256:384],
                                     kt[:, :, 256:512], perf_mode=DR,
                                     start=st, stop=sp)
                    nc.tensor.matmul(gb[:, 384:512], kt[:, :, 384:512],
                                     kt[:, :, 384:512], perf_mode=DR,
                                     start=False, stop=sp)

            ft_all = const.tile([128, 4, R], bf16)

            def emit_tail(b):
                ga, gb, gc = gtiles.pop(b)
                # Gsb full rows [128, row k, d2] from triangle + transposes;
                # copies split across scalar and vector so they drain in
                # parallel under batch b+1's Gram.
                gsb = gpool.tile([128, 4, DKV], bf16, tag="gsb")
                nc.scalar.copy(gsb[:, 0, :], ga[:, :])
                nc.vector.tensor_copy(gsb[:, 1, 128:512], gb[:, 0:384])
                nc.scalar.copy(gsb[:, 2, 256:512], gc[:, 0:256])
                nc.vector.tensor_copy(gsb[:, 3, 384:512], gb[:, 384:512])
                # all 6 mirror transposes packed into one PSUM tile so they
                # run back-to-back on the PE (pending-zero start/stop trick),
                # grouped by destination row so they copy back in 3 strided
                # vector copies. Row-3 mirrors first: FT j=2,1,0 need them.
                tps = pss.tile([128, 6, 128], bf16, tag="scratch")
                for i, (src_row, src_col) in enumerate(
                        ((0, 3), (1, 3), (2, 3), (0, 2), (1, 2), (0, 1))):
                    nc.tensor.matmul(
                        tps[:, i, :],
                        gsb[:, src_row, 128 * src_col:128 * (src_col + 1)],
                        ident[:], is_transpose=True,
                        start=(i == 0), stop=(i == 5))
                nc.vector.tensor_copy(gsb[:, 3, 0:384], tps[:, 0:3, :])
                nc.vector.tensor_copy(gsb[:, 2, 0:256], tps[:, 3:5, :])
                nc.vector.tensor_copy(gsb[:, 1, 0:128], tps[:, 5, :])

                # FT[d2, i] = sum_d1 G[d1, d2] qkT[d1, i]   (G symmetric)
                # j=3 only reads upper blocks, so it runs before the mirror
                # copies land; j=2,1,0 in order of increasing mirror deps.
                ftp = pss.tile([128, 4, SQ], f32, tag="scratch")
                for j in (3, 2, 1, 0):
                    for k in range(4):
                        nc.tensor.matmul(ftp[:, j, :],
                                         gsb[:, k, 128 * j:128 * (j + 1)],
                                         qkT[:, k, SQ * b:SQ * (b + 1)],
                                         start=(k == 0), stop=(k == 3))
                nc.vector.tensor_copy(ft_all[:, :, SQ * b:SQ * (b + 1)], ftp[:])

            # out[(b,i), q] = sum_dk FT[dk, (b,i)] WvT[dk, q], in two halves
            # so the first half's matmul + DMA hide under later Gram work
            # and only the second half sits in the drain.
            out_sb = qpool.tile([R, DQ], f32)

            def emit_out(r0, r1):
                rows = slice(r0, r1)
                outp = pss.tile([r1 - r0, DQ], f32, tag="scratch")
                for j in range(4):
                    nc.tensor.matmul(outp[:], ft_all[:, j, rows],
                                     wv_sb[:, j, :],
                                     start=(j == 0), stop=(j == 3))
                nc.vector.tensor_copy(out_sb[rows, :], outp[:])
                nc.sync.dma_start(out_d[rows, :], out_sb[rows, :])

            # software pipeline: Gram b+1 runs on the PE while batch b's
            # G copies drain on the vector engine.
            emit_gram(0)
            for b in range(BL):
                if b + 1 < BL:
                    emit_gram(b + 1)
                if b == 0:
                    emit_qproj()
                emit_tail(b)
                if b == 4:
                    emit_out(0, 32)
            emit_out(32, 64)

    nc.compile()
    return nc


def _get_nc():
    if "nc" not in _CACHE:
        _CACHE["nc"] = _build()
    return _CACHE["nc"]


def _prep_inputs(q_real, kv, freq_cos, freq_sin, ln_w, ln_b, Wk, Wv):
    f32 = np.float32
    bf16 = ml_dtypes.bfloat16
    f8 = ml_dtypes.float8_e4m3
    q_real = np.asarray(q_real, f32)
    kv = np.asarray(kv, f32)
    freq_cos = np.asarray(freq_cos, f32)
    freq_sin = np.asarray(freq_sin, f32)
    ln_w = np.asarray(ln_w, f32)
    ln_b = np.asarray(ln_b, f32)
    Wk = np.asarray(Wk, f32)
    Wv = np.asarray(Wv, f32)

    # interleaved cos/sin patterns with rotation signs folded in
    C = np.empty((SQ, DQ), f32)
    C[:, 0::2] = freq_cos
    C[:, 1::2] = freq_cos
    S = np.empty((SQ, DQ), f32)
    S[:, 0::2] = -freq_sin
    S[:, 1::2] = freq_sin
    C = np.tile(C, (BL, 1))
    S = np.tile(S, (BL, 1))

    # pair-swapped q (pure layout shuffle; rotation math runs on device)
    qsw = np.empty_like(q_real)
    qsw[..., 0::2] = q_real[..., 1::2]
    qsw[..., 1::2] = q_real[..., 0::2]

    # fold ln_w and the 1/sqrt(dq) score scale into Wk; ln_b becomes a bias
    wk_f = (ln_w[:, None] * Wk) * SCALE           # (256, 512)
    bk = (ln_b @ Wk) * SCALE                      # (512,)
    wk_arr = np.ascontiguousarray(
        wk_f.reshape(2, 128, DKV).transpose(1, 0, 2)).astype(bf16)
    bk_arr = np.ascontiguousarray(bk.reshape(4, 128).T).astype(f32)
    wv_arr = np.ascontiguousarray(
        Wv.T.reshape(4, 128, DQ).transpose(1, 0, 2)).astype(bf16)

    # fp8 kv, permuted so each DMA group is one contiguous block with 4KB
    # partition lines: s = ((g*GCH + gs)*2 + two)*128 + p. The Gram sums
    # over all s, so any fixed permutation of s is exact.
    kv8 = kv.astype(f8)
    kv8 = kv8.reshape(B, NG, GCH, 2, 128, DKV).transpose(0, 1, 4, 2, 3, 5)
    kv8 = np.ascontiguousarray(kv8)               # (B, NG, 128, GCH, 2, DKV)

    in_maps = []
    for c in range(NCORES):
        sl = slice(BL * c, BL * (c + 1))
        in_maps.append({
            "q": np.ascontiguousarray(q_real[sl].reshape(R, DQ)),
            "qsw": np.ascontiguousarray(qsw[sl].reshape(R, DQ)),
            "kv": kv8[sl],
            "cosf": C,
            "sinf": S,
            "wk": wk_arr,
            "bk": bk_arr,
            "wv": wv_arr,
        })
    return in_maps


def kernel(**inputs):
    from concourse.bass_utils import run_bass_kernel_spmd

    nc = _get_nc()
    in_maps = _prep_inputs(**inputs)
    res = run_bass_kernel_spmd(nc, in_maps, list(range(NCORES)))
    out = np.concatenate(
        [res.results[c]["out"].reshape(BL, SQ, DQ) for c in range(NCORES)], axis=0)
    return np.ascontiguousarray(out.astype(np.float32))
